# revision 1
# baseline (speedup 1.0000x reference)
"""Trainium2 Bass kernel for nn_AttentionBlock (B=4, S=2048, D=1024, H=16, Dh=64).

Sharding: 8 cores = 4 batches x 2 head-groups (8 heads each). Every core runs
the same Bass program on different input slices. The output projection is
row-sharded over head-groups, so the host sums the two partial outputs per
batch (the "all-reduce" of the sharding hint, done on host since we return
full outputs anyway).

Per-core pipeline (all matmuls fp32r = full-rate fp32):
  A) QKV projection: lhsT = X^T chunks [128,128], rhs = Wqkv [128,1536 cols]
     -> psum [128(S-tile), 512] per q/k/v. RoPE applied on DVE in
     [S, (h,Dh)] layout, then PE-transposed per head into qT/kT [Dh, S]
     packs. V goes to SBUF augmented with a ones column (V_aug [Sk,65]).
  B) Attention per head: scores computed PRE-TRANSPOSED
     sT[Sk-tile 128, Sq 512] = kT_tile.T @ qT_group. Causal mask added via a
     PE matmul with (-1e30*I) @ mask01 constants. exp on ScalarE
     (PSUM->SBUF). AV: x_aug^T[65, Sq] += V_aug_tile.T @ pT, where row 64
     accumulates the softmax denominator Z for free. Normalization happens
     after a small PE transpose (reciprocal + per-partition scale), then a
     transpose back into xT packs for the output projection.
  C) Output projection: out[Sq,512] += xT_pair.T @ WoutPair, DMA to HBM.
"""

import sys

for _p in ("/opt/pypackages", "/opt/trn_rl_repo"):
    if _p not in sys.path:
        sys.path.insert(0, _p)

import numpy as np
import ml_dtypes

BF16 = ml_dtypes.bfloat16

B, S, D, H, Dh = 4, 2048, 1024, 16, 64
HL = H // 2          # heads per core
NCORES = 8
ST = S // 128        # 16 S-tiles of 128
NG = S // 512        # 4 q-groups of 512
MAX_WAVELENGTH = 10000.0

_CACHE = {}


def _build_bass():
    import concourse.bass as bass
    import concourse.mybir as mybir
    from concourse import bacc
    from concourse.tile import TileContext
    from contextlib import ExitStack

    f32 = mybir.dt.float32
    bf16 = mybir.dt.bfloat16
    AT = mybir.ActivationFunctionType
    OP = mybir.AluOpType

    nc = bacc.Bacc("TRN2", target_bir_lowering=False)

    xt_d = nc.dram_tensor("xt", [D, S], bf16, kind="ExternalInput")
    wqkv_d = nc.dram_tensor("wqkv", [D, 3 * HL * Dh], bf16, kind="ExternalInput")
    wout_d = nc.dram_tensor("woutp", [4, 128, D], bf16, kind="ExternalInput")
    cos_d = nc.dram_tensor("cost", [S, Dh // 2], f32, kind="ExternalInput")
    sin_d = nc.dram_tensor("sint", [S, Dh // 2], f32, kind="ExternalInput")
    negid_d = nc.dram_tensor("negid", [128, 128], bf16, kind="ExternalInput")
    maska_d = nc.dram_tensor("maska", [4, 128, 512], bf16, kind="ExternalInput")
    identb_d = nc.dram_tensor("identb", [128, 128], bf16, kind="ExternalInput")
    identf_d = nc.dram_tensor("ident", [128, 128], f32, kind="ExternalInput")
    ones65_d = nc.dram_tensor("ones65", [1, 65], f32, kind="ExternalInput")
    sel2_d = nc.dram_tensor("sel2", [2, 128], f32, kind="ExternalInput")
    out_d = nc.dram_tensor("out", [S, D], f32, kind="ExternalOutput")

    with TileContext(nc) as tc, ExitStack() as ctx:
        consts = ctx.enter_context(tc.tile_pool(name="consts", bufs=1))
        persist = ctx.enter_context(tc.tile_pool(name="persist", bufs=1))

        identb_sb = consts.tile([128, 128], bf16, tag="identb")
        nc.sync.dma_start(identb_sb, identb_d[:, :])
        identf_sb = consts.tile([128, 128], f32, tag="identf")
        nc.sync.dma_start(identf_sb, identf_d[:, :])
        ones65_sb = consts.tile([1, 65], f32, tag="ones65")
        nc.sync.dma_start(ones65_sb, ones65_d[:, :])
        sel2_sb = consts.tile([2, 128], f32, tag="sel2")
        nc.sync.dma_start(sel2_sb, sel2_d[:, :])
        cos_sb = consts.tile([128, ST, 32], f32, tag="cos")
        nc.sync.dma_start(cos_sb, cos_d.rearrange("(t p) f -> p t f", p=128))
        sin_sb = consts.tile([128, ST, 32], f32, tag="sin")
        nc.sync.dma_start(sin_sb, sin_d.rearrange("(t p) f -> p t f", p=128))
        negid_sb = consts.tile([128, 128], bf16, tag="negid")
        nc.sync.dma_start(negid_sb, negid_d[:, :])
        maska_sb = consts.tile([128, 4, 512], bf16, tag="maska")
        nc.sync.dma_start(maska_sb, maska_d.rearrange("v p n -> p v n"))
        wout_sb = consts.tile([128, 4, 1024], bf16, tag="wout")
        nc.sync.dma_start(wout_sb, wout_d.rearrange("q p n -> p q n"))
        wq_sb = consts.tile([128, 8, 1536], bf16, tag="wqkv")
        nc.sync.dma_start(wq_sb, wqkv_d.rearrange("(c p) n -> p c n", p=128))
        xt_full = consts.tile([128, 8, S], bf16, tag="xtf")
        nc.sync.dma_start(xt_full, xt_d.rearrange("(c p) s -> p c s", p=128))

        qT = persist.tile([128, 4, S], bf16, tag="qT")
        kT = persist.tile([128, 4, S], bf16, tag="kT")
        xT = persist.tile([128, 4, S], bf16, tag="xT")
        vaug = persist.tile([128, HL, ST, Dh + 1], bf16, tag="vaug")
        nc.scalar.activation(
            vaug[:, :, :, Dh : Dh + 1],
            cos_sb[:, 0:1, 0:1, None].to_broadcast((128, HL, ST, 1)),
            AT.Identity, bias=1.0, scale=0.0,
        )

        rw_pool = ctx.enter_context(tc.tile_pool(name="ropew", bufs=3))
        pt_pool = ctx.enter_context(tc.tile_pool(name="ptp", bufs=6))
        nrm_pool = ctx.enter_context(tc.tile_pool(name="nrm", bufs=2))
        xus_pool = ctx.enter_context(tc.tile_pool(name="xus", bufs=4))
        out_pool = ctx.enter_context(tc.tile_pool(name="outp", bufs=3))
        psQ = ctx.enter_context(tc.tile_pool(name="psQ", bufs=3, space="PSUM"))
        psS = ctx.enter_context(tc.tile_pool(name="psS", bufs=3, space="PSUM"))
        psX = ctx.enter_context(tc.tile_pool(name="psX", bufs=1, space="PSUM"))
        psN = ctx.enter_context(tc.tile_pool(name="psN", bufs=1, space="PSUM"))

        def a_chunk(si):
            cos_b = cos_sb[:, si, None, :].to_broadcast((128, HL, 32))
            sin_b = sin_sb[:, si, None, :].to_broadcast((128, HL, 32))
            for qkv, dstT in ((0, qT), (1, kT)):
                ps = psQ.tile([128, 512], f32, tag="pqkv")
                for c in range(8):
                    nc.tensor.matmul(
                        ps, xt_full[:, c, si * 128 : (si + 1) * 128],
                        wq_sb[:, c, qkv * 512 : qkv * 512 + 512],
                        start=(c == 0), stop=(c == 7),
                    )
                v3 = ps.rearrange("p (h d) -> p h d", h=HL)
                x1, x2 = v3[:, :, 0:32], v3[:, :, 32:64]
                rot = rw_pool.tile([128, HL, Dh], bf16, tag="rot")
                t1 = rw_pool.tile([128, HL, 32], f32, tag="t1")
                t2 = rw_pool.tile([128, HL, 32], f32, tag="t2")
                nc.vector.tensor_tensor(t1, x1, cos_b, OP.mult)
                nc.vector.tensor_tensor(t2, x2, sin_b, OP.mult)
                nc.vector.tensor_tensor(rot[:, :, 0:32], t1, t2, OP.subtract)
                nc.vector.tensor_tensor(t1, x1, sin_b, OP.mult)
                nc.vector.tensor_tensor(t2, x2, cos_b, OP.mult)
                nc.vector.tensor_tensor(rot[:, :, 32:64], t1, t2, OP.add)
                rotf = rot.rearrange("p h d -> p (h d)")
                ps_t = psQ.tile([128, 512], f32, tag="pqkv")
                for j in range(4):
                    nc.tensor.matmul(
                        ps_t[:, j * 128 : (j + 1) * 128],
                        rotf[:, j * 128 : (j + 1) * 128],
                        identb_sb, start=True, stop=True,
                    )
                nc.vector.tensor_copy(
                    dstT[:, :, si * 128 : (si + 1) * 128],
                    ps_t.rearrange("p (j s) -> p j s", j=4),
                )
            ps_v = psQ.tile([128, 512], f32, tag="pqkv")
            for c in range(8):
                nc.tensor.matmul(
                    ps_v, xt_full[:, c, si * 128 : (si + 1) * 128],
                    wq_sb[:, c, 1024:1536],
                    start=(c == 0), stop=(c == 7),
                )
            nc.vector.tensor_copy(
                vaug[:, :, si, 0:Dh],
                ps_v.rearrange("p (h d) -> p h d", h=HL),
            )

        def b_chunk(g):
            zsb8 = nrm_pool.tile([HL, 512], f32, tag="zsb8")
            xus_l = []
            nj = 4 * (g + 1)
            for h in range(HL):
                hp, hh = h % 2, h // 2
                kslice = kT[64 * hp : 64 * hp + 64, hh, :]
                qg = qT[64 * hp : 64 * hp + 64, hh, g * 512 : (g + 1) * 512]
                ps_x = psX.tile([Dh + 1, 512], f32, tag="psx")
                for j in range(nj):
                    diag = j >= 4 * g
                    c0 = 128 * (j - 4 * g) if diag else 0
                    ps_s = psS.tile([128, 512], f32, tag="pss")
                    nc.tensor.matmul(
                        ps_s[:, c0:512],
                        kslice[:, j * 128 : (j + 1) * 128],
                        qg[:, c0:512],
                        start=True,
                        stop=not diag,
                    )
                    if diag:
                        v = j - 4 * g
                        nc.tensor.matmul(
                            ps_s[:, c0:512], negid_sb,
                            maska_sb[:, v, c0:512],
                            start=False, stop=True,
                        )
                    pt = pt_pool.tile([128, 512], bf16, tag="pt")
                    nc.scalar.activation(pt[:, c0:512], ps_s[:, c0:512], AT.Exp)
                    nc.tensor.matmul(
                        ps_x[:, c0:512], vaug[:, h, j, :], pt[:, c0:512],
                        start=(j == 0), stop=(j == nj - 1),
                    )
                zt = nrm_pool.tile([1, 512], f32, tag="zt")
                nc.scalar.copy(zt, ps_x[Dh : Dh + 1, :])
                nc.gpsimd.dma_start(zsb8[h : h + 1, :], zt)
                if hp == 0:
                    xus_l.append(xus_pool.tile([128, 512], bf16, tag="xus", name="xus"))
                nc.vector.tensor_copy(
                    xus_l[hh][64 * hp : 64 * hp + 64, :], ps_x[0:Dh, :]
                )
            # batched normalization: Z rows -> cols -> recip -> pair rows ->
            # pair broadcast -> per-head multiply into xT
            zc_ps = psN.tile([128, 4, HL], f32, tag="psn")
            for m in range(4):
                nc.tensor.matmul(
                    zc_ps[:, m, :],
                    zsb8[:, m * 128 : (m + 1) * 128],
                    identf_sb[0:HL, 0:HL],
                    start=True, stop=True,
                )
            rcol8 = nrm_pool.tile([128, 4, HL], f32, tag="rcol")
            nc.vector.reciprocal(rcol8, zc_ps)
            for pj in range(4):
                zrp_ps = psN.tile([2, 512], f32, tag="psn")
                for m in range(4):
                    nc.tensor.matmul(
                        zrp_ps[:, m * 128 : (m + 1) * 128],
                        rcol8[:, m, 2 * pj : 2 * pj + 2],
                        identf_sb,
                        start=True, stop=True,
                    )
                zrowp = nrm_pool.tile([2, 512], f32, tag="zrowp")
                nc.scalar.copy(zrowp, zrp_ps)
                bc_ps = psN.tile([128, 512], f32, tag="psn")
                nc.tensor.matmul(bc_ps, sel2_sb, zrowp, start=True, stop=True)
                bcsp = nrm_pool.tile([128, 512], f32, tag="bcsp")
                nc.scalar.copy(bcsp, bc_ps)
                nc.vector.tensor_tensor(
                    xT[:, pj, g * 512 : (g + 1) * 512],
                    xus_l[pj],
                    bcsp,
                    OP.mult,
                )

        def c_chunk(m):
            for half in range(2):
                ps_o = psS.tile([128, 512], f32, tag="pss")
                for p in range(4):
                    nc.tensor.matmul(
                        ps_o,
                        xT[:, p, m * 128 : (m + 1) * 128],
                        wout_sb[:, p, half * 512 : (half + 1) * 512],
                        start=(p == 0),
                        stop=(p == 3),
                    )
                ob = out_pool.tile([128, 512], f32, tag="ob")
                nc.vector.tensor_copy(ob, ps_o)
                nc.sync.dma_start(
                    out_d[m * 128 : (m + 1) * 128,
                          half * 512 : (half + 1) * 512],
                    ob,
                )

        for g in range(NG):
            for si in range(4 * g, 4 * g + 4):
                a_chunk(si)
            b_chunk(g)
            if g >= 1:
                for m in range(4 * (g - 1), 4 * g):
                    c_chunk(m)
        for m in range(4 * (NG - 1), S // 128):
            c_chunk(m)

    nc.compile()
    return nc


def _numpy_fallback(x, w_q, w_k, w_v, w_out, seg, mask):
    """Exact numpy replica of the reference for non-causal masks."""
    frac = (2.0 * np.arange(Dh // 2, dtype=np.float32)) / Dh
    ts = (MAX_WAVELENGTH ** frac).astype(np.float32)

    def rope(t, pos):
        sinu = pos.astype(np.float32)[:, :, None] / ts  # [B,S,32]
        sn, cs = np.sin(sinu), np.cos(sinu)
        sn, cs = sn[:, :, None, :], cs[:, :, None, :]
        f, s_ = t[..., :32], t[..., 32:]
        return np.concatenate([f * cs - s_ * sn, s_ * cs + f * sn], -1)

    q = np.einsum("bsd,dhk->bshk", x, w_q)
    k = np.einsum("bsd,dhk->bshk", x, w_k)
    v = np.einsum("bsd,dhk->bshk", x, w_v)
    q, k = rope(q, seg), rope(k, seg)
    q = q / np.sqrt(np.float32(Dh))
    attn = np.einsum("bqhd,bkhd->bhqk", q, k)
    attn = np.where(mask, attn, np.finfo(np.float32).min)
    attn = attn - attn.max(-1, keepdims=True)
    e = np.exp(attn)
    attn = e / e.sum(-1, keepdims=True)
    xo = np.einsum("bhqk,bkhd->bqhd", attn, v)
    return np.einsum("bqhd,hdm->bqm", xo, w_out).astype(np.float32)


def _host_inputs(x, w_q, w_k, w_v, w_out, seg):
    frac = (2.0 * np.arange(Dh // 2, dtype=np.float32)) / Dh
    ts = (MAX_WAVELENGTH ** frac).astype(np.float32)
    negid = (np.eye(128, dtype=np.float32) * -1e30).astype(BF16)
    ident = np.eye(128, dtype=np.float32)
    identb = np.eye(128, dtype=np.float32).astype(BF16)
    ones65 = np.ones((1, 65), dtype=np.float32)
    sel2 = np.zeros((2, 128), dtype=np.float32)
    sel2[0, 0:64] = 1.0
    sel2[1, 64:128] = 1.0
    rr = np.arange(128)[:, None]
    cc = np.arange(512)[None, :]
    maska = np.stack(
        [(rr + 128 * v > cc).astype(BF16) for v in range(4)]
    )  # [4,128,512], 1 where masked

    in_maps = []
    for core in range(NCORES):
        b, g = core // 2, core % 2
        hs = slice(g * HL, (g + 1) * HL)
        wq_s = (w_q[:, hs, :] / np.float32(np.sqrt(Dh))).reshape(D, HL * Dh)
        wk_s = w_k[:, hs, :].reshape(D, HL * Dh)
        wv_s = w_v[:, hs, :].reshape(D, HL * Dh)
        wqkv = np.ascontiguousarray(
            np.concatenate([wq_s, wk_s, wv_s], axis=1), dtype=np.float32
        ).astype(BF16)
        woutp = np.stack(
            [
                w_out[g * HL + 2 * p : g * HL + 2 * p + 2].reshape(128, D)
                for p in range(4)
            ]
        ).astype(BF16)
        sinu = seg[b].astype(np.float32)[:, None] / ts  # [S, 32]
        in_maps.append(
            {
                "xt": np.ascontiguousarray(x[b].T).astype(BF16),
                "wqkv": wqkv,
                "woutp": np.ascontiguousarray(woutp),
                "cost": np.cos(sinu).astype(np.float32),
                "sint": np.sin(sinu).astype(np.float32),
                "negid": negid,
                "maska": np.ascontiguousarray(maska),
                "ident": ident,
                "identb": identb,
                "ones65": ones65,
                "sel2": sel2,
            }
        )
    return in_maps


def _run(in_maps, trace=False):
    from concourse.bass_utils import run_bass_kernel_spmd

    if "nc" not in _CACHE:
        _CACHE["nc"] = _build_bass()
    return run_bass_kernel_spmd(
        _CACHE["nc"], in_maps, core_ids=list(range(NCORES)), trace=trace
    )


def kernel(**inputs):
    x = np.asarray(inputs["inputs"], dtype=np.float32)
    w_q = np.asarray(inputs["w_q"], dtype=np.float32)
    w_k = np.asarray(inputs["w_k"], dtype=np.float32)
    w_v = np.asarray(inputs["w_v"], dtype=np.float32)
    w_out = np.asarray(inputs["w_out"], dtype=np.float32)
    seg = np.asarray(inputs["segment_positions"])
    mask = np.asarray(inputs["mask"])

    causal = np.tril(np.ones((S, S), dtype=bool))
    if not all(np.array_equal(mask[b, 0], causal) for b in range(B)):
        return _numpy_fallback(x, w_q, w_k, w_v, w_out, seg, mask)

    in_maps = _host_inputs(x, w_q, w_k, w_v, w_out, seg)
    res = _run(in_maps)
    outs = [r_["out"] for r_ in res.results]
    result = np.empty((B, S, D), dtype=np.float32)
    for b in range(B):
        result[b] = outs[2 * b] + outs[2 * b + 1]
    return result

